# revision 1
# baseline (speedup 1.0000x reference)
"""MiniGPT (2-layer causal transformer + tied-embedding logits) on 8 trn2 cores.

Sharding:
  - Attention: tensor-parallel over heads (2 heads/core). Each core computes
    q,k,v projections for its 2 heads, causal-blocked attention in "ST"
    (scores-transposed) layout, and the normalized per-head combined output
    (128 of the 1024 combined dims). An AllGather concatenates the per-core
    combined slices, then every core applies the full output projection and
    residual locally (identical on all cores, avoiding an AllReduce).
  - Logits: vocab-sharded tied-embedding matmul. Core r holds rows
    [r*VP, (r+1)*VP) of the (zero-padded) token embedding and emits
    logits[:, r*VP:(r+1)*VP]; host concatenates and trims padding.

All matmuls run in bf16 (fp32 PSUM accumulation); softmax runs in fp32
(scores are tiny, ~1e-4, so exp needs no max-subtraction; masked entries
are exactly zeroed by a multiplicative 0/1 mask after exp).

Activations live transposed (xT: [hidden, seq]) the whole time, which makes
every matmul PE-friendly with no on-device transposes at all.
"""

import os as _os
import sys as _sys

if "jax" not in _sys.modules and _os.environ.get("JAX_PLATFORMS") == "cpu":
    # bass2jax needs the axon trn2 devices; a cpu pin would hide them
    del _os.environ["JAX_PLATFORMS"]

import numpy as np
import ml_dtypes

import concourse.bass as bass
import concourse.mybir as mybir
import concourse.tile as tile
from concourse import bacc
from concourse.bass import ts, ds
from concourse.bass_utils import run_bass_kernel_spmd

P = 128
H = 1024
DT = H // P  # 8 hidden-dim tiles
NH = 16
HD = 64
V = 50259
NCORES = 8
S_FULL = 2048
VP_FULL = -(-V // NCORES)  # 6283 per-core padded vocab shard

F32 = mybir.dt.float32
BF16 = mybir.dt.bfloat16
BF = ml_dtypes.bfloat16


def build_nc(S, VP, no_collective=False, skip_layers=False, skip_logits=False):
    """Build the per-core Bass program (SPMD: same NEFF on all 8 cores).

    no_collective=True replaces the AllGather with local DMA block copies
    (single-core cost-model profiling only - numerically wrong)."""
    ST = S // P       # seq tiles of 128
    SC = S // 512     # seq chunks of 512
    NVC = -(-VP // 512)  # vocab chunks

    nc = bacc.Bacc("TRN2", target_bir_lowering=False, debug=False,
                   num_devices=NCORES)

    # --- DRAM I/O (per-core) ---
    x0f = nc.dram_tensor("x0f", [H, S], F32, kind="ExternalInput")
    x0b = nc.dram_tensor("x0b", [H, S], BF16, kind="ExternalInput")
    pe = nc.dram_tensor("pe", [H, S], BF16, kind="ExternalInput")
    x0pe = nc.dram_tensor("x0pe", [H, S], BF16, kind="ExternalInput")
    wqkv = nc.dram_tensor("wqkv", [2, H, 3 * P], BF16, kind="ExternalInput")
    wo = nc.dram_tensor("wo", [2, H, P], BF16, kind="ExternalInput")
    et = nc.dram_tensor("et", [H, VP], BF16, kind="ExternalInput")
    msk = nc.dram_tensor("msk", [4, P, 512], BF16, kind="ExternalInput")
    out = nc.dram_tensor("logits", [S, VP], F32, kind="ExternalOutput")

    Exp = mybir.ActivationFunctionType.Exp

    with tile.TileContext(nc) as tc:
        with (
            tc.tile_pool(name="const", bufs=1) as const,
            tc.tile_pool(name="dram", bufs=1, space="DRAM") as dram,
        ):
            # persistent SBUF tensors
            xT = const.tile([P, DT, S], F32, tag="xT")      # fp32 residual
            xb = const.tile([P, DT, S], BF16, tag="xb")     # bf16 copy of x
            maskt = const.tile([P, 4, 512], BF16, tag="maskt")
            et0 = const.tile([P, DT, 512], BF16, tag="et0")
            et0_w = min(512, VP)


            with (
                tc.tile_pool(name="big", bufs=1) as big,
                tc.tile_pool(name="lay", bufs=1) as lay,
                tc.tile_pool(name="pestream", bufs=4) as pestream,
                tc.tile_pool(name="xpepool", bufs=4) as xpepool,
                tc.tile_pool(name="epool", bufs=6) as epool,
                tc.tile_pool(name="bcpool", bufs=2) as bcpool,
                tc.tile_pool(name="ps_a", bufs=4, space="PSUM") as ps_a,
                tc.tile_pool(name="ps_pv", bufs=2, space="PSUM") as ps_pv,
            ):
                wq_tiles = {}
                pe0 = lay.tile([P, DT, 512], BF16, tag="pe0")

                def load_wq(l):
                    t = lay.tile([P, DT, 3 * P], BF16, tag=f"wqkv{l}")
                    nc.sync.dma_start(
                        t[:], wqkv[l].rearrange("(d p) e -> p d e", p=P)
                    )
                    wq_tiles[l] = t

                load_wq(0)
                for layer in range(0 if skip_layers else 2):
                    wq = wq_tiles[layer]

                    # fused q,k projections over streamed xpe=(xb+pe) tiles
                    qT = lay.tile([P, S], BF16, tag="qT")
                    kT = lay.tile([P, S], BF16, tag="kT")
                    for c in range(SC):
                        psQ = ps_a.tile([P, 512], F32, tag="mm")
                        psK = ps_a.tile([P, 512], F32, tag="mm")
                        for d in range(DT):
                            xpet = xpepool.tile([P, 512], BF16, tag="xpet")
                            if layer == 0:
                                nc.sync.dma_start(
                                    xpet[:],
                                    x0pe[:].rearrange(
                                        "(d p) s -> p d s", p=P
                                    )[:, d, ts(c, 512)],
                                )
                            else:
                                if c == 0:
                                    pet = pe0[:, d]
                                else:
                                    pet = pestream.tile(
                                        [P, 512], BF16, tag="pet"
                                    )
                                    nc.sync.dma_start(
                                        pet[:],
                                        pe[:].rearrange(
                                            "(d p) s -> p d s", p=P
                                        )[:, d, ts(c, 512)],
                                    )
                                nc.vector.tensor_add(
                                    xpet[:], xb[:, d, ts(c, 512)], pet[:]
                                )
                            nc.tensor.matmul(
                                psQ[:], wq[:, d, ts(0, P)], xpet[:],
                                start=(d == 0), stop=(d == DT - 1),
                                skip_group_check=True,
                            )
                            nc.tensor.matmul(
                                psK[:], wq[:, d, ts(1, P)], xpet[:],
                                start=(d == 0), stop=(d == DT - 1),
                                skip_group_check=True,
                            )
                        nc.vector.tensor_copy(qT[:, ts(c, 512)], psQ[:])
                        nc.vector.tensor_copy(kT[:, ts(c, 512)], psK[:])

                    if layer == 0:
                        # xb load deferred: q/k proj reads x0pe, so this
                        # streams in under the projection matmuls
                        for d in range(DT):
                            nc.sync.dma_start(
                                xb[:, d],
                                x0b[:].rearrange("(d p) s -> p d s", p=P)[:, d],
                            )

                    # v projection -> natural layout [ks, vdim] with ones cols:
                    # v[:, st, 0:65]  = [vA(64) | 1]   (head A lhsT)
                    # v[:, st, 65:130]= [vB(64) | 1]   (head B lhsT)
                    v = lay.tile([P, ST, 130], BF16, tag="v")
                    nc.vector.memset(v[:, :, 64:65], 1.0)
                    nc.vector.memset(v[:, :, 129:130], 1.0)
                    for st in range(ST):
                        psum = ps_a.tile([P, 512], F32, tag="mm")
                        for d in range(DT):
                            nc.tensor.matmul(
                                psum[:, :P],
                                xb[:, d, ts(st, P)],
                                wq[:, d, 2 * P : 3 * P],
                                start=(d == 0),
                                stop=(d == DT - 1),
                            )
                        nc.vector.tensor_copy(v[:, st, 0:64], psum[:, 0:64])
                        nc.vector.tensor_copy(
                            v[:, st, 65:129], psum[:, 64:128]
                        )

                    if layer == 0:
                        # deferred loads: masks are first needed by the
                        # attention exps, the fp32 residual baseline by the
                        # layer-0 residual add - keep them off the startup
                        # DMA critical path
                        nc.sync.dma_start(
                            maskt[:], msk[:].rearrange("o p n -> p o n")
                        )
                        nc.sync.dma_start(
                            xT[:], x0f[:].rearrange("(d p) s -> p d s", p=P)
                        )

                    # causal attention, ST layout, 2 heads packed on partitions
                    # comb layout [64, 2, S]: [:,0]=head A dims, [:,1]=head B
                    comb = lay.tile([64, 2, S], BF16, tag="comb")
                    ag_outs = []

                    def ag_emit(qc, comb=comb):
                        agin = dram.tile([P, 512], BF16, tag=f"agin{qc}")
                        agout = dram.tile(
                            [NCORES * P, 512], BF16, tag=f"agout{qc}",
                            addr_space="Local" if no_collective else "Shared",
                        )
                        nc.sync.dma_start(
                            agin[:].rearrange("(h p) s -> p h s", p=64),
                            comb[:, :, ts(qc, 512)],
                        )
                        if no_collective:
                            for r in range(NCORES):
                                nc.sync.dma_start(
                                    agout[r * P : (r + 1) * P, :], agin[:]
                                )
                        else:
                            nc.gpsimd.collective_compute(
                                "AllGather",
                                mybir.AluOpType.bypass,
                                ins=[agin[:]],
                                outs=[agout[:]],
                                replica_groups=[list(range(NCORES))],
                            )
                        ag_outs.append(agout)

                    for qc in range(SC):
                        n_kt = 4 * (qc + 1)
                        pvA = ps_pv.tile([65, 512], F32, tag="pvA")
                        pvB = ps_pv.tile([65, 512], F32, tag="pvB")
                        for kt in range(n_kt):
                            sA = ps_a.tile([P, 512], F32, tag="mm")
                            sB = ps_a.tile([P, 512], F32, tag="mm")
                            nc.tensor.matmul(
                                sA[:], kT[0:64, ts(kt, P)],
                                qT[0:64, ts(qc, 512)],
                                start=True, stop=True,
                            )
                            nc.tensor.matmul(
                                sB[:], kT[64:P, ts(kt, P)],
                                qT[64:P, ts(qc, 512)],
                                start=True, stop=True,
                            )
                            eA = epool.tile([P, 512], BF16, tag="e")
                            eB = epool.tile([P, 512], BF16, tag="e")
                            nc.scalar.activation(eA[:], sA[:], Exp, scale=0.125)
                            nc.scalar.activation(eB[:], sB[:], Exp, scale=0.125)
                            o = kt - 4 * qc
                            if o >= 0:
                                nc.vector.tensor_mul(eA[:], eA[:], maskt[:, o])
                                nc.vector.tensor_mul(eB[:], eB[:], maskt[:, o])
                            nc.tensor.matmul(
                                pvA[:], v[:, kt, 0:65], eA[:],
                                start=(kt == 0), stop=(kt == n_kt - 1),
                                skip_group_check=True,
                            )
                            nc.tensor.matmul(
                                pvB[:], v[:, kt, 65:130], eB[:],
                                start=(kt == 0), stop=(kt == n_kt - 1),
                                skip_group_check=True,
                            )
                        # normalize: pv psum row 64 is the softmax denominator
                        bcA = bcpool.tile([64, 512], F32, tag="bcA")
                        bcB = bcpool.tile([64, 512], F32, tag="bcB")
                        nc.vector.reciprocal(bcA[0:1, :], pvA[64:65, :])
                        nc.vector.reciprocal(bcB[0:1, :], pvB[64:65, :])
                        nc.gpsimd.partition_broadcast(bcA[:], bcA[0:1, :])
                        nc.gpsimd.partition_broadcast(bcB[:], bcB[0:1, :])
                        nc.vector.tensor_mul(
                            comb[:, 0, ts(qc, 512)], pvA[0:64, :], bcA[:]
                        )
                        nc.vector.tensor_mul(
                            comb[:, 1, ts(qc, 512)], pvB[0:64, :], bcB[:]
                        )
                        ag_emit(qc)

                    # per-chunk AllGather (issued inside the qc loop via
                    # ag_emit) so comm overlaps remaining attention chunks
                    agouts = ag_outs
                    # output projection, sharded by OUTPUT columns: this
                    # core computes only its 128 e-dims (wo input carries the
                    # matching weight slice), then a per-chunk bf16 AllGather
                    # of the residual delta reassembles the full update on
                    # every core. 32 matmuls/layer instead of 256.
                    woL = lay.tile([P, DT, P], BF16, tag="woL")
                    nc.sync.dma_start(
                        woL[:], wo[layer].rearrange("(d p) e -> p d e", p=P)
                    )
                    d_outs = []
                    for c in range(SC):
                        cb = big.tile([P, DT, 512], BF16, tag=f"cb{c}")
                        nc.sync.dma_start(
                            cb[:],
                            agouts[c][:].rearrange("(d p) s -> p d s", p=P),
                        )
                        psum = ps_a.tile([P, 512], F32, tag="mm")
                        for d in range(DT):
                            nc.tensor.matmul(
                                psum[:],
                                woL[:, d, :],
                                cb[:, d, :],
                                start=(d == 0),
                                stop=(d == DT - 1),
                            )
                        dsb = bcpool.tile([P, 512], BF16, tag="dsb")
                        nc.vector.tensor_copy(dsb[:], psum[:])
                        d_in = dram.tile([P, 512], BF16, tag=f"d2in{c}")
                        d_out = dram.tile(
                            [NCORES * P, 512], BF16, tag=f"d2out{c}",
                            addr_space="Local" if no_collective else "Shared",
                        )
                        nc.sync.dma_start(d_in[:], dsb[:])
                        if no_collective:
                            for r in range(NCORES):
                                nc.sync.dma_start(
                                    d_out[r * P : (r + 1) * P, :], d_in[:]
                                )
                        else:
                            nc.gpsimd.collective_compute(
                                "AllGather",
                                mybir.AluOpType.bypass,
                                ins=[d_in[:]],
                                outs=[d_out[:]],
                                replica_groups=[list(range(NCORES))],
                            )
                        d_outs.append(d_out)

                    if layer == 0:
                        # prefetches issued while the SP queue would otherwise
                        # idle at the AG wait (everything issued later sits
                        # behind the delta-AG sem waits - in-order queue):
                        # first logits embedding tile, layer-1 qkv weights,
                        # and layer-1's first pe chunk
                        nc.sync.dma_start(
                            et0[:, :, :et0_w],
                            et[:].rearrange("(d p) v -> p d v", p=P)[
                                :, :, ds(0, et0_w)
                            ],
                        )
                        load_wq(1)
                        nc.sync.dma_start(
                            pe0[:],
                            pe[:].rearrange("(d p) s -> p d s", p=P)[
                                :, :, ts(0, 512)
                            ],
                        )

                    # residual update from the gathered full delta
                    for c in range(SC):
                        cb2 = big.tile([P, DT, 512], BF16, tag=f"cb{c}")
                        nc.sync.dma_start(
                            cb2[:],
                            d_outs[c][:].rearrange("(d p) s -> p d s", p=P),
                        )
                        for ei in range(DT):
                            nc.vector.tensor_add(
                                xT[:, ei, ts(c, 512)], xT[:, ei, ts(c, 512)],
                                cb2[:, ei, :],
                            )
                            nc.scalar.copy(
                                xb[:, ei, ts(c, 512)], xT[:, ei, ts(c, 512)]
                            )

            # logits: xb.T @ E_shard.T, vocab-chunked
            with (
                tc.tile_pool(name="etpool", bufs=3) as etpool,
                tc.tile_pool(name="opool", bufs=4) as opool,
                tc.tile_pool(name="ps_l", bufs=8, space="PSUM") as ps_l,
            ):
                for vc in range(0 if skip_logits else NVC):
                    w = min(512, VP - vc * 512)
                    if vc == 0:
                        ett = et0
                    else:
                        ett = etpool.tile([P, DT, 512], BF16, tag="et")
                        nc.sync.dma_start(
                            ett[:, :, :w],
                            et[:].rearrange("(d p) v -> p d v", p=P)[
                                :, :, ds(vc * 512, w)
                            ],
                        )
                    for st in range(ST):
                        psum = ps_l.tile([P, 512], F32, tag="lmm")
                        for d in range(DT):
                            nc.tensor.matmul(
                                psum[:, :w],
                                xb[:, d, ts(st, P)],
                                ett[:, d, :w],
                                start=(d == 0),
                                stop=(d == DT - 1),
                            )
                        ot = opool.tile([P, 512], F32, tag="o")
                        nc.vector.tensor_copy(ot[:, :w], psum[:, :w])
                        nc.sync.dma_start(
                            out[ts(st, P), ds(vc * 512, w)], ot[:, :w]
                        )

    nc.compile()
    return nc


def make_host_inputs(input_ids, tok_emb, pos_emb, Wqkvs, Wos, S, VP):
    """Shard/transpose/cast inputs on host. Returns per-core in_maps."""
    ids = np.asarray(input_ids).reshape(-1)[:S]
    tok = np.asarray(tok_emb, dtype=np.float32)
    pos = np.asarray(pos_emb, dtype=np.float32)

    x0 = tok[ids]                                  # (S, H) fp32
    x0f = np.ascontiguousarray(x0.T)               # (H, S)
    x0b = x0f.astype(BF)
    peT = np.ascontiguousarray(pos[:S].T).astype(BF)
    x0peT = np.ascontiguousarray((x0 + pos[:S]).T).astype(BF)

    # causal 0/1 masks in ST layout for the 4 diagonal-region offsets
    msk = np.zeros((4, P, 512), np.float32)
    tri = (np.arange(P)[:, None] <= np.arange(P)[None, :]).astype(np.float32)
    for o in range(4):
        for j in range(4):
            blk = msk[o, :, j * P : (j + 1) * P]
            if j > o:
                blk[:] = 1.0
            elif j == o:
                blk[:] = tri
    msk = msk.astype(BF)

    # padded vocab shards of the (transposed) embedding
    tokP = np.zeros((NCORES * VP, H), np.float32)
    tokP[:V] = tok

    in_maps = []
    for r in range(NCORES):
        wq_l = []
        for Wqkv in Wqkvs:
            Wq = Wqkv[r * P : (r + 1) * P]
            Wk = Wqkv[H + r * P : H + (r + 1) * P]
            Wv = Wqkv[2 * H + r * P : 2 * H + (r + 1) * P]
            wq_l.append(np.concatenate([Wq, Wk, Wv], axis=0).T)  # (H, 384)
        wqkv_arr = np.stack(wq_l).astype(BF)
        wo_arr = np.stack([np.asarray(Wo).T[:, r * P : (r + 1) * P] for Wo in Wos]).astype(BF)
        eT = np.ascontiguousarray(tokP[r * VP : (r + 1) * VP].T).astype(BF)
        in_maps.append(
            {
                "x0f": x0f,
                "x0b": x0b,
                "pe": peT,
                "x0pe": x0peT,
                "wqkv": np.ascontiguousarray(wqkv_arr),
                "wo": np.ascontiguousarray(wo_arr),
                "et": eT,
                "msk": msk,
            }
        )
    return in_maps


_NC_CACHE = {}


def _get_nc(S, VP):
    key = (S, VP)
    if key not in _NC_CACHE:
        _NC_CACHE[key] = build_nc(S, VP)
    return _NC_CACHE[key]


class _Runner:
    """Compiled 8-core PJRT executable for one Bass program.

    Mirrors bass2jax.run_bass_via_pjrt's multi-core path but keeps the
    jitted function so repeated calls (benchmarking) skip recompilation.
    """

    def __init__(self, nc):
        import jax
        import jax.numpy as jnp
        from jax.sharding import Mesh, PartitionSpec, NamedSharding
        from jax.experimental.shard_map import shard_map
        from concourse import bass2jax
        from concourse import mybir as _mybir

        bass2jax.install_neuronx_cc_hook()
        self.jax = jax
        self.jnp = jnp
        self.nc = nc

        partition_name = (
            nc.partition_id_tensor.name if nc.partition_id_tensor else None
        )
        in_names, out_names, out_avals = [], [], []
        for alloc in nc.m.functions[0].allocations:
            if not isinstance(alloc, _mybir.MemoryLocationSet):
                continue
            name = alloc.memorylocations[0].name
            if alloc.kind == "ExternalInput":
                if name != partition_name:
                    in_names.append(name)
            elif alloc.kind == "ExternalOutput":
                out_names.append(name)
                out_avals.append(
                    jax.core.ShapedArray(
                        tuple(alloc.tensor_shape), _mybir.dt.np(alloc.dtype)
                    )
                )
        n_params = len(in_names)
        all_in_names = list(in_names) + list(out_names)
        if partition_name is not None:
            all_in_names.append(partition_name)
        self.in_names = in_names
        self.out_names = out_names
        self.out_avals = out_avals

        def _body(*args):
            operands = list(args)
            if partition_name is not None:
                operands.append(bass2jax.partition_id_tensor())
            outs = bass2jax._bass_exec_p.bind(
                *operands,
                out_avals=tuple(out_avals),
                in_names=tuple(all_in_names),
                out_names=tuple(out_names),
                lowering_input_output_aliases=(),
                sim_require_finite=True,
                sim_require_nnan=True,
                nc=nc,
            )
            return tuple(outs)

        devices = jax.devices()[:NCORES]
        self.mesh = Mesh(np.asarray(devices), ("core",))
        self.sharding = NamedSharding(self.mesh, PartitionSpec("core"))
        n_outs = len(out_names)
        donate = tuple(range(n_params, n_params + n_outs))
        self.fn = jax.jit(
            shard_map(
                _body,
                mesh=self.mesh,
                in_specs=(PartitionSpec("core"),) * (n_params + n_outs),
                out_specs=(PartitionSpec("core"),) * n_outs,
                check_rep=False,
            ),
            donate_argnums=donate,
            keep_unused=True,
        )
        self.dev_inputs = None

    def stage_inputs(self, in_maps):
        """device_put concatenated per-core inputs (reusable across calls)."""
        concat = [
            np.concatenate([np.asarray(m[n]) for m in in_maps], axis=0)
            for n in self.in_names
        ]
        self.dev_inputs = [
            self.jax.device_put(a, self.sharding) for a in concat
        ]

    def _zeros(self):
        if not hasattr(self, "_zeros_fn"):
            jnp = self.jnp
            shapes = [
                ((NCORES * av.shape[0],) + tuple(av.shape[1:]), av.dtype)
                for av in self.out_avals
            ]
            self._zeros_fn = self.jax.jit(
                lambda: tuple(jnp.zeros(s, d) for s, d in shapes),
                out_shardings=tuple(self.sharding for _ in shapes),
            )
        outs = self._zeros_fn()
        self.jax.block_until_ready(outs)
        return outs

    def run(self, zeros=None):
        if zeros is None:
            zeros = self._zeros()
        out = self.fn(*self.dev_inputs, *zeros)
        self.jax.block_until_ready(out)
        return out

    def results(self, out_arrs):
        per_core = []
        for c in range(NCORES):
            d = {}
            for i, name in enumerate(self.out_names):
                a = np.asarray(out_arrs[i])
                d[name] = a.reshape((NCORES,) + self.out_avals[i].shape)[c]
            per_core.append(d)
        return per_core


_RUNNER_CACHE = {}


def _get_runner(S, VP):
    key = (S, VP)
    if key not in _RUNNER_CACHE:
        _RUNNER_CACHE[key] = _Runner(_get_nc(S, VP))
    return _RUNNER_CACHE[key]


def run_model(input_ids, tok_emb, pos_emb, Wqkvs, Wos, S=S_FULL, VP=VP_FULL,
              bench_iters=0):
    import time as _time

    runner = _get_runner(S, VP)
    in_maps = make_host_inputs(input_ids, tok_emb, pos_emb, Wqkvs, Wos, S, VP)
    runner.stage_inputs(in_maps)
    out = runner.run()
    times = []
    for _ in range(bench_iters):
        zs = runner._zeros()
        t0 = _time.perf_counter()
        out = runner.run(zeros=zs)
        times.append(_time.perf_counter() - t0)
    res = runner.results(out)
    logits = np.concatenate(
        [res[r]["logits"] for r in range(NCORES)], axis=1
    )[:, : min(V, NCORES * VP)]
    return logits, times


def kernel(**inputs):
    logits, _ = run_model(
        inputs["input_ids"],
        inputs["tok_emb"],
        inputs["pos_emb"],
        [np.asarray(inputs["Wqkv0"], np.float32),
         np.asarray(inputs["Wqkv1"], np.float32)],
        [np.asarray(inputs["Wo0"], np.float32),
         np.asarray(inputs["Wo1"], np.float32)],
    )
    return logits[None].astype(np.float32)



# revision 9
# speedup vs baseline: 1.0122x; 1.0122x over previous
"""MiniGPT (2-layer causal transformer + tied-embedding logits) on 8 trn2 cores.

Sharding:
  - Attention: tensor-parallel over heads (2 heads/core). Each core computes
    q,k,v projections for its 2 heads, causal-blocked attention in "ST"
    (scores-transposed) layout, and the normalized per-head combined output
    (128 of the 1024 combined dims). A per-chunk AllGather concatenates the
    per-core combined slices; every core then applies the FULL output
    projection and residual locally (redundant compute, but it removes the
    second per-layer collective round - collectives serialize on the
    collective engine at ~15us fixed cost each, so 4 AGs/layer beats 8).
  - Logits: vocab-sharded tied-embedding matmul. Core r holds rows
    [r*VP, (r+1)*VP) of the (zero-padded) token embedding and emits
    logits[:, r*VP:(r+1)*VP]; host concatenates and trims padding.

All matmuls run in bf16 (fp32 PSUM accumulation); softmax runs in fp32
(scores are tiny, ~1e-4, so exp needs no max-subtraction; masked entries
are exactly zeroed by a multiplicative 0/1 mask after exp).

Activations live transposed (xT: [hidden, seq]) the whole time, which makes
every matmul PE-friendly with no on-device transposes at all.
"""

import os as _os
import sys as _sys

if "jax" not in _sys.modules and _os.environ.get("JAX_PLATFORMS") == "cpu":
    # bass2jax needs the axon trn2 devices; a cpu pin would hide them
    del _os.environ["JAX_PLATFORMS"]

import numpy as np
import ml_dtypes

import concourse.bass as bass
import concourse.mybir as mybir
import concourse.tile as tile
from concourse import bacc
from concourse.bass import ts, ds
from concourse.bass_utils import run_bass_kernel_spmd

P = 128
H = 1024
DT = H // P  # 8 hidden-dim tiles
NH = 16
HD = 64
V = 50259
NCORES = 8
S_FULL = 2048
VP_FULL = -(-V // NCORES)  # 6283 per-core padded vocab shard

F32 = mybir.dt.float32
BF16 = mybir.dt.bfloat16
BF = ml_dtypes.bfloat16


def build_nc(S, VP, no_collective=False, skip_layers=False, skip_logits=False):
    """Build the per-core Bass program (SPMD: same NEFF on all 8 cores).

    no_collective=True replaces the AllGather with local DMA block copies
    (single-core cost-model profiling only - numerically wrong)."""
    ST = S // P       # seq tiles of 128
    SC = S // 512     # seq chunks of 512
    NVC = -(-VP // 512)  # vocab chunks

    nc = bacc.Bacc("TRN2", target_bir_lowering=False, debug=False,
                   num_devices=NCORES)

    # --- DRAM I/O (per-core) ---
    x0f = nc.dram_tensor("x0f", [H, S], F32, kind="ExternalInput")
    x0b = nc.dram_tensor("x0b", [H, S], BF16, kind="ExternalInput")
    pe = nc.dram_tensor("pe", [H, S], BF16, kind="ExternalInput")
    x0pe = nc.dram_tensor("x0pe", [H, S], BF16, kind="ExternalInput")
    wqkv = nc.dram_tensor("wqkv", [2, H, 3 * P], BF16, kind="ExternalInput")
    wo = nc.dram_tensor("wo", [2, H, H], BF16, kind="ExternalInput")
    et = nc.dram_tensor("et", [H, VP], BF16, kind="ExternalInput")
    msk = nc.dram_tensor("msk", [4, P, 512], BF16, kind="ExternalInput")
    out = nc.dram_tensor("logits", [S, VP], F32, kind="ExternalOutput")

    Exp = mybir.ActivationFunctionType.Exp

    with tile.TileContext(nc) as tc:
        with (
            tc.tile_pool(name="const", bufs=1) as const,
            tc.tile_pool(name="dram", bufs=1, space="DRAM") as dram,
        ):
            # persistent SBUF tensors
            xT = const.tile([P, DT, S], F32, tag="xT")      # fp32 residual
            xb = const.tile([P, DT, S], BF16, tag="xb")     # bf16 copy of x
            maskt = const.tile([P, 4, 512], BF16, tag="maskt")
            et0 = const.tile([P, DT, 512], BF16, tag="et0")
            et0_w = min(512, VP)


            with (
                tc.tile_pool(name="big", bufs=2) as big,
                tc.tile_pool(name="lay", bufs=1) as lay,
                tc.tile_pool(name="pestream", bufs=4) as pestream,
                tc.tile_pool(name="xpepool", bufs=4) as xpepool,
                tc.tile_pool(name="epool", bufs=6) as epool,
                tc.tile_pool(name="bcpool", bufs=2) as bcpool,
                tc.tile_pool(name="ps_a", bufs=4, space="PSUM") as ps_a,
                tc.tile_pool(name="ps_pv", bufs=2, space="PSUM") as ps_pv,
            ):
                wq_tiles = {}
                pe0 = lay.tile([P, DT, 512], BF16, tag="pe0")

                def load_wq(l):
                    t = lay.tile([P, DT, 3 * P], BF16, tag=f"wqkv{l}")
                    nc.sync.dma_start(
                        t[:], wqkv[l].rearrange("(d p) e -> p d e", p=P)
                    )
                    wq_tiles[l] = t

                load_wq(0)
                for layer in range(0 if skip_layers else 2):
                    wq = wq_tiles[layer]

                    # fused q,k projections over streamed xpe=(xb+pe) tiles
                    qT = lay.tile([P, S], BF16, tag="qT")
                    kT = lay.tile([P, S], BF16, tag="kT")
                    for c in range(SC):
                        psQ = ps_a.tile([P, 512], F32, tag="mm")
                        psK = ps_a.tile([P, 512], F32, tag="mm")
                        for d in range(DT):
                            xpet = xpepool.tile([P, 512], BF16, tag="xpet")
                            if layer == 0:
                                nc.sync.dma_start(
                                    xpet[:],
                                    x0pe[:].rearrange(
                                        "(d p) s -> p d s", p=P
                                    )[:, d, ts(c, 512)],
                                )
                            else:
                                if c == 0:
                                    pet = pe0[:, d]
                                else:
                                    pet = pestream.tile(
                                        [P, 512], BF16, tag="pet"
                                    )
                                    nc.sync.dma_start(
                                        pet[:],
                                        pe[:].rearrange(
                                            "(d p) s -> p d s", p=P
                                        )[:, d, ts(c, 512)],
                                    )
                                nc.vector.tensor_add(
                                    xpet[:], xb[:, d, ts(c, 512)], pet[:]
                                )
                            nc.tensor.matmul(
                                psQ[:], wq[:, d, ts(0, P)], xpet[:],
                                start=(d == 0), stop=(d == DT - 1),
                                skip_group_check=True,
                            )
                            nc.tensor.matmul(
                                psK[:], wq[:, d, ts(1, P)], xpet[:],
                                start=(d == 0), stop=(d == DT - 1),
                                skip_group_check=True,
                            )
                        nc.vector.tensor_copy(qT[:, ts(c, 512)], psQ[:])
                        nc.vector.tensor_copy(kT[:, ts(c, 512)], psK[:])

                    if layer == 0:
                        # xb load deferred: q/k proj reads x0pe, so this
                        # streams in under the projection matmuls
                        for d in range(DT):
                            nc.sync.dma_start(
                                xb[:, d],
                                x0b[:].rearrange("(d p) s -> p d s", p=P)[:, d],
                            )

                    # v projection -> natural layout [ks, vdim] with ones cols:
                    # v[:, st, 0:65]  = [vA(64) | 1]   (head A lhsT)
                    # v[:, st, 65:130]= [vB(64) | 1]   (head B lhsT)
                    v = lay.tile([P, ST, 130], BF16, tag="v")
                    nc.vector.memset(v[:, :, 64:65], 1.0)
                    nc.vector.memset(v[:, :, 129:130], 1.0)
                    for st in range(ST):
                        psum = ps_a.tile([P, 512], F32, tag="mm")
                        for d in range(DT):
                            nc.tensor.matmul(
                                psum[:, :P],
                                xb[:, d, ts(st, P)],
                                wq[:, d, 2 * P : 3 * P],
                                start=(d == 0),
                                stop=(d == DT - 1),
                            )
                        nc.vector.tensor_copy(v[:, st, 0:64], psum[:, 0:64])
                        nc.vector.tensor_copy(
                            v[:, st, 65:129], psum[:, 64:128]
                        )

                    # full WoT for this layer (2MB): issued here so it
                    # streams in under the attention chunks; first needed at
                    # the output projection
                    woL = lay.tile([P, DT, H], BF16, tag="woL")
                    nc.sync.dma_start(
                        woL[:], wo[layer].rearrange("(d p) e -> p d e", p=P)
                    )

                    if layer == 0:
                        # deferred loads: masks are first needed by the
                        # attention exps, the fp32 residual baseline by the
                        # layer-0 residual add - keep them off the startup
                        # DMA critical path; xT is chunked so chunk 0 lands
                        # before the first residual add
                        nc.sync.dma_start(
                            maskt[:], msk[:].rearrange("o p n -> p o n")
                        )
                        for c in range(SC):
                            nc.sync.dma_start(
                                xT[:, :, ts(c, 512)],
                                x0f[:].rearrange("(d p) s -> p d s", p=P)[
                                    :, :, ts(c, 512)
                                ],
                            )

                    # causal attention, ST layout, 2 heads packed on partitions
                    # comb layout [64, 2, S]: [:,0]=head A dims, [:,1]=head B
                    comb = lay.tile([64, 2, S], BF16, tag="comb")
                    ag_outs = []

                    def ag_emit(qc, comb=comb):
                        agin = dram.tile([P, 512], BF16, tag=f"agin{qc}")
                        agout = dram.tile(
                            [NCORES * P, 512], BF16, tag=f"agout{qc}",
                            addr_space="Local" if no_collective else "Shared",
                        )
                        nc.sync.dma_start(
                            agin[:].rearrange("(h p) s -> p h s", p=64),
                            comb[:, :, ts(qc, 512)],
                        )
                        if no_collective:
                            for r in range(NCORES):
                                nc.sync.dma_start(
                                    agout[r * P : (r + 1) * P, :], agin[:]
                                )
                        else:
                            nc.gpsimd.collective_compute(
                                "AllGather",
                                mybir.AluOpType.bypass,
                                ins=[agin[:]],
                                outs=[agout[:]],
                                replica_groups=[list(range(NCORES))],
                            )
                        ag_outs.append(agout)

                    for qc in range(SC):
                        n_kt = 4 * (qc + 1)
                        pvA = ps_pv.tile([65, 512], F32, tag="pvA")
                        pvB = ps_pv.tile([65, 512], F32, tag="pvB")
                        for kt in range(n_kt):
                            sA = ps_a.tile([P, 512], F32, tag="mm")
                            sB = ps_a.tile([P, 512], F32, tag="mm")
                            nc.tensor.matmul(
                                sA[:], kT[0:64, ts(kt, P)],
                                qT[0:64, ts(qc, 512)],
                                start=True, stop=True,
                            )
                            nc.tensor.matmul(
                                sB[:], kT[64:P, ts(kt, P)],
                                qT[64:P, ts(qc, 512)],
                                start=True, stop=True,
                            )
                            eA = epool.tile([P, 512], BF16, tag="e")
                            eB = epool.tile([P, 512], BF16, tag="e")
                            nc.scalar.activation(eA[:], sA[:], Exp, scale=0.125)
                            nc.scalar.activation(eB[:], sB[:], Exp, scale=0.125)
                            o = kt - 4 * qc
                            if o >= 0:
                                nc.vector.tensor_mul(eA[:], eA[:], maskt[:, o])
                                nc.vector.tensor_mul(eB[:], eB[:], maskt[:, o])
                            nc.tensor.matmul(
                                pvA[:], v[:, kt, 0:65], eA[:],
                                start=(kt == 0), stop=(kt == n_kt - 1),
                                skip_group_check=True,
                            )
                            nc.tensor.matmul(
                                pvB[:], v[:, kt, 65:130], eB[:],
                                start=(kt == 0), stop=(kt == n_kt - 1),
                                skip_group_check=True,
                            )
                        # normalize: pv psum row 64 is the softmax denominator
                        bcA = bcpool.tile([64, 512], F32, tag="bcA")
                        bcB = bcpool.tile([64, 512], F32, tag="bcB")
                        nc.vector.reciprocal(bcA[0:1, :], pvA[64:65, :])
                        nc.vector.reciprocal(bcB[0:1, :], pvB[64:65, :])
                        nc.gpsimd.partition_broadcast(bcA[:], bcA[0:1, :])
                        nc.gpsimd.partition_broadcast(bcB[:], bcB[0:1, :])
                        nc.vector.tensor_mul(
                            comb[:, 0, ts(qc, 512)], pvA[0:64, :], bcA[:]
                        )
                        nc.vector.tensor_mul(
                            comb[:, 1, ts(qc, 512)], pvB[0:64, :], bcB[:]
                        )
                        ag_emit(qc)

                    # per-chunk AllGather (issued inside the qc loop via
                    # ag_emit) so comm overlaps remaining attention chunks
                    agouts = ag_outs

                    if layer == 0:
                        # prefetches issued while the SP queue would otherwise
                        # idle at the comb-AG waits (in-order queue): first
                        # logits embedding tile, layer-1 qkv weights, and
                        # layer-1's first pe chunk
                        nc.sync.dma_start(
                            et0[:, :, :et0_w],
                            et[:].rearrange("(d p) v -> p d v", p=P)[
                                :, :, ds(0, et0_w)
                            ],
                        )
                        load_wq(1)
                        nc.sync.dma_start(
                            pe0[:],
                            pe[:].rearrange("(d p) s -> p d s", p=P)[
                                :, :, ts(0, 512)
                            ],
                        )

                    # full output projection on every core (redundant across
                    # cores but collective-free: the comb AG already gave
                    # every core the full combined tensor, so computing all
                    # 1024 e-dims locally replaces the delta-AllGather round
                    # trip), fused with the residual update per 512-chunk
                    for c in range(SC):
                        cb = big.tile([P, DT, 512], BF16, tag="cb")
                        nc.sync.dma_start(
                            cb[:],
                            agouts[c][:].rearrange("(d p) s -> p d s", p=P),
                        )
                        for e in range(DT):
                            psum = ps_a.tile([P, 512], F32, tag="mm")
                            for d in range(DT):
                                nc.tensor.matmul(
                                    psum[:],
                                    woL[:, d, ts(e, P)],
                                    cb[:, d, :],
                                    start=(d == 0),
                                    stop=(d == DT - 1),
                                )
                            nc.vector.tensor_add(
                                xT[:, e, ts(c, 512)], xT[:, e, ts(c, 512)],
                                psum[:],
                            )
                            nc.scalar.copy(
                                xb[:, e, ts(c, 512)], xT[:, e, ts(c, 512)]
                            )

            # logits: xb.T @ E_shard.T, vocab-chunked
            with (
                tc.tile_pool(name="etpool", bufs=3) as etpool,
                tc.tile_pool(name="opool", bufs=4) as opool,
                tc.tile_pool(name="ps_l", bufs=8, space="PSUM") as ps_l,
            ):
                for vc in range(0 if skip_logits else NVC):
                    w = min(512, VP - vc * 512)
                    if vc == 0:
                        ett = et0
                    else:
                        ett = etpool.tile([P, DT, 512], BF16, tag="et")
                        nc.sync.dma_start(
                            ett[:, :, :w],
                            et[:].rearrange("(d p) v -> p d v", p=P)[
                                :, :, ds(vc * 512, w)
                            ],
                        )
                    for st in range(ST):
                        psum = ps_l.tile([P, 512], F32, tag="lmm")
                        for d in range(DT):
                            nc.tensor.matmul(
                                psum[:, :w],
                                xb[:, d, ts(st, P)],
                                ett[:, d, :w],
                                start=(d == 0),
                                stop=(d == DT - 1),
                            )
                        ot = opool.tile([P, 512], F32, tag="o")
                        nc.vector.tensor_copy(ot[:, :w], psum[:, :w])
                        nc.sync.dma_start(
                            out[ts(st, P), ds(vc * 512, w)], ot[:, :w]
                        )

    nc.compile()
    return nc


def make_host_inputs(input_ids, tok_emb, pos_emb, Wqkvs, Wos, S, VP):
    """Shard/transpose/cast inputs on host. Returns per-core in_maps."""
    ids = np.asarray(input_ids).reshape(-1)[:S]
    tok = np.asarray(tok_emb, dtype=np.float32)
    pos = np.asarray(pos_emb, dtype=np.float32)

    x0 = tok[ids]                                  # (S, H) fp32
    x0f = np.ascontiguousarray(x0.T)               # (H, S)
    x0b = x0f.astype(BF)
    peT = np.ascontiguousarray(pos[:S].T).astype(BF)
    x0peT = np.ascontiguousarray((x0 + pos[:S]).T).astype(BF)

    # causal 0/1 masks in ST layout for the 4 diagonal-region offsets
    msk = np.zeros((4, P, 512), np.float32)
    tri = (np.arange(P)[:, None] <= np.arange(P)[None, :]).astype(np.float32)
    for o in range(4):
        for j in range(4):
            blk = msk[o, :, j * P : (j + 1) * P]
            if j > o:
                blk[:] = 1.0
            elif j == o:
                blk[:] = tri
    msk = msk.astype(BF)

    # padded vocab shards of the (transposed) embedding
    tokP = np.zeros((NCORES * VP, H), np.float32)
    tokP[:V] = tok

    in_maps = []
    for r in range(NCORES):
        wq_l = []
        for Wqkv in Wqkvs:
            Wq = Wqkv[r * P : (r + 1) * P]
            Wk = Wqkv[H + r * P : H + (r + 1) * P]
            Wv = Wqkv[2 * H + r * P : 2 * H + (r + 1) * P]
            wq_l.append(np.concatenate([Wq, Wk, Wv], axis=0).T)  # (H, 384)
        wqkv_arr = np.stack(wq_l).astype(BF)
        wo_arr = np.stack([np.asarray(Wo).T for Wo in Wos]).astype(BF)
        eT = np.ascontiguousarray(tokP[r * VP : (r + 1) * VP].T).astype(BF)
        in_maps.append(
            {
                "x0f": x0f,
                "x0b": x0b,
                "pe": peT,
                "x0pe": x0peT,
                "wqkv": np.ascontiguousarray(wqkv_arr),
                "wo": np.ascontiguousarray(wo_arr),
                "et": eT,
                "msk": msk,
            }
        )
    return in_maps


_NC_CACHE = {}


def _get_nc(S, VP):
    key = (S, VP)
    if key not in _NC_CACHE:
        _NC_CACHE[key] = build_nc(S, VP)
    return _NC_CACHE[key]


class _Runner:
    """Compiled 8-core PJRT executable for one Bass program.

    Mirrors bass2jax.run_bass_via_pjrt's multi-core path but keeps the
    jitted function so repeated calls (benchmarking) skip recompilation.
    """

    def __init__(self, nc):
        import jax
        import jax.numpy as jnp
        from jax.sharding import Mesh, PartitionSpec, NamedSharding
        from jax.experimental.shard_map import shard_map
        from concourse import bass2jax
        from concourse import mybir as _mybir

        bass2jax.install_neuronx_cc_hook()
        self.jax = jax
        self.jnp = jnp
        self.nc = nc

        partition_name = (
            nc.partition_id_tensor.name if nc.partition_id_tensor else None
        )
        in_names, out_names, out_avals = [], [], []
        for alloc in nc.m.functions[0].allocations:
            if not isinstance(alloc, _mybir.MemoryLocationSet):
                continue
            name = alloc.memorylocations[0].name
            if alloc.kind == "ExternalInput":
                if name != partition_name:
                    in_names.append(name)
            elif alloc.kind == "ExternalOutput":
                out_names.append(name)
                out_avals.append(
                    jax.core.ShapedArray(
                        tuple(alloc.tensor_shape), _mybir.dt.np(alloc.dtype)
                    )
                )
        n_params = len(in_names)
        all_in_names = list(in_names) + list(out_names)
        if partition_name is not None:
            all_in_names.append(partition_name)
        self.in_names = in_names
        self.out_names = out_names
        self.out_avals = out_avals

        def _body(*args):
            operands = list(args)
            if partition_name is not None:
                operands.append(bass2jax.partition_id_tensor())
            outs = bass2jax._bass_exec_p.bind(
                *operands,
                out_avals=tuple(out_avals),
                in_names=tuple(all_in_names),
                out_names=tuple(out_names),
                lowering_input_output_aliases=(),
                sim_require_finite=True,
                sim_require_nnan=True,
                nc=nc,
            )
            return tuple(outs)

        devices = jax.devices()[:NCORES]
        self.mesh = Mesh(np.asarray(devices), ("core",))
        self.sharding = NamedSharding(self.mesh, PartitionSpec("core"))
        n_outs = len(out_names)
        donate = tuple(range(n_params, n_params + n_outs))
        self.fn = jax.jit(
            shard_map(
                _body,
                mesh=self.mesh,
                in_specs=(PartitionSpec("core"),) * (n_params + n_outs),
                out_specs=(PartitionSpec("core"),) * n_outs,
                check_rep=False,
            ),
            donate_argnums=donate,
            keep_unused=True,
        )
        self.dev_inputs = None

    def stage_inputs(self, in_maps):
        """device_put concatenated per-core inputs (reusable across calls)."""
        concat = [
            np.concatenate([np.asarray(m[n]) for m in in_maps], axis=0)
            for n in self.in_names
        ]
        self.dev_inputs = [
            self.jax.device_put(a, self.sharding) for a in concat
        ]

    def _zeros(self):
        if not hasattr(self, "_zeros_fn"):
            jnp = self.jnp
            shapes = [
                ((NCORES * av.shape[0],) + tuple(av.shape[1:]), av.dtype)
                for av in self.out_avals
            ]
            self._zeros_fn = self.jax.jit(
                lambda: tuple(jnp.zeros(s, d) for s, d in shapes),
                out_shardings=tuple(self.sharding for _ in shapes),
            )
        outs = self._zeros_fn()
        self.jax.block_until_ready(outs)
        return outs

    def run(self, zeros=None):
        if zeros is None:
            zeros = self._zeros()
        out = self.fn(*self.dev_inputs, *zeros)
        self.jax.block_until_ready(out)
        return out

    def results(self, out_arrs):
        per_core = []
        for c in range(NCORES):
            d = {}
            for i, name in enumerate(self.out_names):
                a = np.asarray(out_arrs[i])
                d[name] = a.reshape((NCORES,) + self.out_avals[i].shape)[c]
            per_core.append(d)
        return per_core


_RUNNER_CACHE = {}


def _get_runner(S, VP):
    key = (S, VP)
    if key not in _RUNNER_CACHE:
        _RUNNER_CACHE[key] = _Runner(_get_nc(S, VP))
    return _RUNNER_CACHE[key]


def run_model(input_ids, tok_emb, pos_emb, Wqkvs, Wos, S=S_FULL, VP=VP_FULL,
              bench_iters=0):
    import time as _time

    runner = _get_runner(S, VP)
    in_maps = make_host_inputs(input_ids, tok_emb, pos_emb, Wqkvs, Wos, S, VP)
    runner.stage_inputs(in_maps)
    out = runner.run()
    times = []
    for _ in range(bench_iters):
        zs = runner._zeros()
        t0 = _time.perf_counter()
        out = runner.run(zeros=zs)
        times.append(_time.perf_counter() - t0)
    res = runner.results(out)
    logits = np.concatenate(
        [res[r]["logits"] for r in range(NCORES)], axis=1
    )[:, : min(V, NCORES * VP)]
    return logits, times


def kernel(**inputs):
    logits, _ = run_model(
        inputs["input_ids"],
        inputs["tok_emb"],
        inputs["pos_emb"],
        [np.asarray(inputs["Wqkv0"], np.float32),
         np.asarray(inputs["Wqkv1"], np.float32)],
        [np.asarray(inputs["Wo0"], np.float32),
         np.asarray(inputs["Wo1"], np.float32)],
    )
    return logits[None].astype(np.float32)



# revision 12
# speedup vs baseline: 1.0161x; 1.0038x over previous
"""MiniGPT (2-layer causal transformer + tied-embedding logits) on 8 trn2 cores.

Sharding:
  - Attention: tensor-parallel over heads (2 heads/core). Each core computes
    q,k,v projections for its 2 heads, causal-blocked attention in "ST"
    (scores-transposed) layout, and the normalized per-head combined output
    (128 of the 1024 combined dims). A per-chunk AllGather concatenates the
    per-core combined slices; every core then applies the FULL output
    projection and residual locally (redundant compute, but it removes the
    second per-layer collective round - collectives serialize on the
    collective engine at ~15us fixed cost each, so 4 AGs/layer beats 8).
  - Logits: vocab-sharded tied-embedding matmul. Core r holds rows
    [r*VP, (r+1)*VP) of the (zero-padded) token embedding and emits
    logits[:, r*VP:(r+1)*VP]; host concatenates and trims padding.

All matmuls run in bf16 (fp32 PSUM accumulation); softmax runs in fp32
(scores are tiny, ~1e-4, so exp needs no max-subtraction; masked entries
are exactly zeroed by a multiplicative 0/1 mask after exp).

Activations live transposed (xT: [hidden, seq]) the whole time, which makes
every matmul PE-friendly with no on-device transposes at all.
"""

import os as _os
import sys as _sys

if "jax" not in _sys.modules and _os.environ.get("JAX_PLATFORMS") == "cpu":
    # bass2jax needs the axon trn2 devices; a cpu pin would hide them
    del _os.environ["JAX_PLATFORMS"]

import numpy as np
import ml_dtypes

import concourse.bass as bass
import concourse.mybir as mybir
import concourse.tile as tile
from concourse import bacc
from concourse.bass import ts, ds
from concourse.bass_utils import run_bass_kernel_spmd

P = 128
H = 1024
DT = H // P  # 8 hidden-dim tiles
NH = 16
HD = 64
V = 50259
NCORES = 8
S_FULL = 2048
VP_FULL = -(-V // NCORES)  # 6283 per-core padded vocab shard

F32 = mybir.dt.float32
BF16 = mybir.dt.bfloat16
BF = ml_dtypes.bfloat16


def build_nc(S, VP, no_collective=False, skip_layers=False, skip_logits=False):
    """Build the per-core Bass program (SPMD: same NEFF on all 8 cores).

    no_collective=True replaces the AllGather with local DMA block copies
    (single-core cost-model profiling only - numerically wrong)."""
    ST = S // P       # seq tiles of 128
    SC = S // 512     # seq chunks of 512
    NVC = -(-VP // 512)  # vocab chunks

    nc = bacc.Bacc("TRN2", target_bir_lowering=False, debug=False,
                   num_devices=NCORES)

    # --- DRAM I/O (per-core) ---
    x0f = nc.dram_tensor("x0f", [H, S], F32, kind="ExternalInput")
    x0b = nc.dram_tensor("x0b", [H, S], BF16, kind="ExternalInput")
    pe = nc.dram_tensor("pe", [H, S], BF16, kind="ExternalInput")
    x0pe = nc.dram_tensor("x0pe", [H, S], BF16, kind="ExternalInput")
    wqkv = nc.dram_tensor("wqkv", [2, H, 3 * P], BF16, kind="ExternalInput")
    wo = nc.dram_tensor("wo", [2, H, H], BF16, kind="ExternalInput")
    et = nc.dram_tensor("et", [H, VP], BF16, kind="ExternalInput")
    msk = nc.dram_tensor("msk", [4, P, 512], BF16, kind="ExternalInput")
    out = nc.dram_tensor("logits", [S, VP], F32, kind="ExternalOutput")

    Exp = mybir.ActivationFunctionType.Exp

    with tile.TileContext(nc) as tc:
        with (
            tc.tile_pool(name="const", bufs=1) as const,
            tc.tile_pool(name="dram", bufs=1, space="DRAM") as dram,
        ):
            # persistent SBUF tensors
            xT = const.tile([P, DT, S], F32, tag="xT")      # fp32 residual
            xb = const.tile([P, DT, S], BF16, tag="xb")     # bf16 copy of x
            maskt = const.tile([P, 4, 512], BF16, tag="maskt")
            et0 = const.tile([P, DT, 512], BF16, tag="et0")
            et0_w = min(512, VP)


            with (
                tc.tile_pool(name="big", bufs=2) as big,
                tc.tile_pool(name="lay", bufs=1) as lay,
                tc.tile_pool(name="pestream", bufs=4) as pestream,
                tc.tile_pool(name="xpepool", bufs=4) as xpepool,
                tc.tile_pool(name="epool", bufs=6) as epool,
                tc.tile_pool(name="bcpool", bufs=2) as bcpool,
                tc.tile_pool(name="ps_a", bufs=4, space="PSUM") as ps_a,
                tc.tile_pool(name="ps_pv", bufs=2, space="PSUM") as ps_pv,
            ):
                wq_tiles = {}
                pe0 = lay.tile([P, DT, 512], BF16, tag="pe0")

                def load_wq(l):
                    t = lay.tile([P, DT, 3 * P], BF16, tag=f"wqkv{l}")
                    nc.sync.dma_start(
                        t[:], wqkv[l].rearrange("(d p) e -> p d e", p=P)
                    )
                    wq_tiles[l] = t

                load_wq(0)
                for layer in range(0 if skip_layers else 2):
                    wq = wq_tiles[layer]

                    # fused q,k projections over streamed xpe=(xb+pe) tiles
                    qT = lay.tile([P, S], BF16, tag="qT")
                    kT = lay.tile([P, S], BF16, tag="kT")
                    for c in range(SC):
                        psQ = ps_a.tile([P, 512], F32, tag="mm")
                        psK = ps_a.tile([P, 512], F32, tag="mm")
                        for d in range(DT):
                            xpet = xpepool.tile([P, 512], BF16, tag="xpet")
                            if layer == 0:
                                nc.sync.dma_start(
                                    xpet[:],
                                    x0pe[:].rearrange(
                                        "(d p) s -> p d s", p=P
                                    )[:, d, ts(c, 512)],
                                )
                            else:
                                if c == 0:
                                    pet = pe0[:, d]
                                else:
                                    pet = pestream.tile(
                                        [P, 512], BF16, tag="pet"
                                    )
                                    nc.sync.dma_start(
                                        pet[:],
                                        pe[:].rearrange(
                                            "(d p) s -> p d s", p=P
                                        )[:, d, ts(c, 512)],
                                    )
                                nc.vector.tensor_add(
                                    xpet[:], xb[:, d, ts(c, 512)], pet[:]
                                )
                            nc.tensor.matmul(
                                psQ[:], wq[:, d, ts(0, P)], xpet[:],
                                start=(d == 0), stop=(d == DT - 1),
                                skip_group_check=True,
                            )
                            nc.tensor.matmul(
                                psK[:], wq[:, d, ts(1, P)], xpet[:],
                                start=(d == 0), stop=(d == DT - 1),
                                skip_group_check=True,
                            )
                        nc.vector.tensor_copy(qT[:, ts(c, 512)], psQ[:])
                        nc.vector.tensor_copy(kT[:, ts(c, 512)], psK[:])

                    if layer == 0:
                        # xb load deferred: q/k proj reads x0pe, so this
                        # streams in under the projection matmuls
                        for d in range(DT):
                            nc.sync.dma_start(
                                xb[:, d],
                                x0b[:].rearrange("(d p) s -> p d s", p=P)[:, d],
                            )

                    # v projection -> natural layout [ks, vdim] with ones cols:
                    # v[:, st, 0:65]  = [vA(64) | 1]   (head A lhsT)
                    # v[:, st, 65:130]= [vB(64) | 1]   (head B lhsT)
                    v = lay.tile([P, ST, 130], BF16, tag="v")
                    nc.vector.memset(v[:, :, 64:65], 1.0)
                    nc.vector.memset(v[:, :, 129:130], 1.0)
                    for st in range(ST):
                        psum = ps_a.tile([P, 512], F32, tag="mm")
                        for d in range(DT):
                            nc.tensor.matmul(
                                psum[:, :P],
                                xb[:, d, ts(st, P)],
                                wq[:, d, 2 * P : 3 * P],
                                start=(d == 0),
                                stop=(d == DT - 1),
                            )
                        nc.vector.tensor_copy(v[:, st, 0:64], psum[:, 0:64])
                        nc.vector.tensor_copy(
                            v[:, st, 65:129], psum[:, 64:128]
                        )

                    # full WoT for this layer (2MB): issued here so it
                    # streams in under the attention chunks; first needed at
                    # the output projection
                    woL = lay.tile([P, DT, H], BF16, tag="woL")
                    nc.sync.dma_start(
                        woL[:], wo[layer].rearrange("(d p) e -> p d e", p=P)
                    )

                    if layer == 0:
                        # deferred loads: masks are first needed by the
                        # attention exps, the fp32 residual baseline by the
                        # layer-0 residual add - keep them off the startup
                        # DMA critical path; xT is chunked so chunk 0 lands
                        # before the first residual add
                        nc.sync.dma_start(
                            maskt[:], msk[:].rearrange("o p n -> p o n")
                        )
                        for c in range(SC):
                            nc.sync.dma_start(
                                xT[:, :, ts(c, 512)],
                                x0f[:].rearrange("(d p) s -> p d s", p=P)[
                                    :, :, ts(c, 512)
                                ],
                            )

                    # causal attention, ST layout, 2 heads packed on partitions
                    # comb layout [64, 2, S]: [:,0]=head A dims, [:,1]=head B
                    comb = lay.tile([64, 2, S], BF16, tag="comb")
                    ag_outs = []

                    def ag_emit(qc, comb=comb):
                        agin = dram.tile([P, 512], BF16, tag=f"agin{qc}")
                        agout = dram.tile(
                            [NCORES * P, 512], BF16, tag=f"agout{qc}",
                            addr_space="Local" if no_collective else "Shared",
                        )
                        nc.sync.dma_start(
                            agin[:].rearrange("(h p) s -> p h s", p=64),
                            comb[:, :, ts(qc, 512)],
                        )
                        if no_collective:
                            for r in range(NCORES):
                                nc.sync.dma_start(
                                    agout[r * P : (r + 1) * P, :], agin[:]
                                )
                        else:
                            nc.gpsimd.collective_compute(
                                "AllGather",
                                mybir.AluOpType.bypass,
                                ins=[agin[:]],
                                outs=[agout[:]],
                                replica_groups=[list(range(NCORES))],
                            )
                        ag_outs.append(agout)

                    for qc in range(SC):
                        n_kt = 4 * (qc + 1)
                        pvA = ps_pv.tile([65, 512], F32, tag="pvA")
                        pvB = ps_pv.tile([65, 512], F32, tag="pvB")
                        for kt in range(n_kt):
                            sA = ps_a.tile([P, 512], F32, tag="mm")
                            sB = ps_a.tile([P, 512], F32, tag="mm")
                            nc.tensor.matmul(
                                sA[:], kT[0:64, ts(kt, P)],
                                qT[0:64, ts(qc, 512)],
                                start=True, stop=True,
                            )
                            nc.tensor.matmul(
                                sB[:], kT[64:P, ts(kt, P)],
                                qT[64:P, ts(qc, 512)],
                                start=True, stop=True,
                            )
                            eA = epool.tile([P, 512], BF16, tag="e")
                            eB = epool.tile([P, 512], BF16, tag="e")
                            nc.scalar.activation(eA[:], sA[:], Exp, scale=0.125)
                            nc.scalar.activation(eB[:], sB[:], Exp, scale=0.125)
                            o = kt - 4 * qc
                            if o >= 0:
                                nc.vector.tensor_mul(eA[:], eA[:], maskt[:, o])
                                nc.vector.tensor_mul(eB[:], eB[:], maskt[:, o])
                            nc.tensor.matmul(
                                pvA[:], v[:, kt, 0:65], eA[:],
                                start=(kt == 0), stop=(kt == n_kt - 1),
                                skip_group_check=True,
                            )
                            nc.tensor.matmul(
                                pvB[:], v[:, kt, 65:130], eB[:],
                                start=(kt == 0), stop=(kt == n_kt - 1),
                                skip_group_check=True,
                            )
                        # normalize: pv psum row 64 is the softmax denominator
                        bcA = bcpool.tile([64, 512], F32, tag="bcA")
                        bcB = bcpool.tile([64, 512], F32, tag="bcB")
                        nc.vector.reciprocal(bcA[0:1, :], pvA[64:65, :])
                        nc.vector.reciprocal(bcB[0:1, :], pvB[64:65, :])
                        nc.gpsimd.partition_broadcast(bcA[:], bcA[0:1, :])
                        nc.gpsimd.partition_broadcast(bcB[:], bcB[0:1, :])
                        nc.vector.tensor_mul(
                            comb[:, 0, ts(qc, 512)], pvA[0:64, :], bcA[:]
                        )
                        nc.vector.tensor_mul(
                            comb[:, 1, ts(qc, 512)], pvB[0:64, :], bcB[:]
                        )
                        ag_emit(qc)

                    # per-chunk AllGather (issued inside the qc loop via
                    # ag_emit) so comm overlaps remaining attention chunks
                    agouts = ag_outs

                    if layer == 0:
                        # prefetches issued while the SP queue would otherwise
                        # idle at the comb-AG waits (in-order queue): first
                        # logits embedding tile, layer-1 qkv weights, and
                        # layer-1's first pe chunk
                        nc.sync.dma_start(
                            et0[:, :, :et0_w],
                            et[:].rearrange("(d p) v -> p d v", p=P)[
                                :, :, ds(0, et0_w)
                            ],
                        )
                        load_wq(1)
                        nc.sync.dma_start(
                            pe0[:],
                            pe[:].rearrange("(d p) s -> p d s", p=P)[
                                :, :, ts(0, 512)
                            ],
                        )

                    # full output projection on every core (redundant across
                    # cores but collective-free: the comb AG already gave
                    # every core the full combined tensor, so computing all
                    # 1024 e-dims locally replaces the delta-AllGather round
                    # trip), fused with the residual update per 512-chunk
                    for c in range(SC):
                        cb = big.tile([P, DT, 512], BF16, tag="cb")
                        nc.sync.dma_start(
                            cb[:],
                            agouts[c][:].rearrange("(d p) s -> p d s", p=P),
                        )
                        for e in range(DT):
                            psum = ps_a.tile([P, 512], F32, tag="mm")
                            for d in range(DT):
                                nc.tensor.matmul(
                                    psum[:],
                                    woL[:, d, ts(e, P)],
                                    cb[:, d, :],
                                    start=(d == 0),
                                    stop=(d == DT - 1),
                                )
                            nc.vector.tensor_add(
                                xT[:, e, ts(c, 512)], xT[:, e, ts(c, 512)],
                                psum[:],
                            )
                            nc.scalar.copy(
                                xb[:, e, ts(c, 512)], xT[:, e, ts(c, 512)]
                            )

            # logits: xb.T @ E_shard.T, vocab-chunked
            with (
                tc.tile_pool(name="etpool", bufs=3) as etpool,
                tc.tile_pool(name="opool", bufs=4) as opool,
                tc.tile_pool(name="ps_l", bufs=8, space="PSUM") as ps_l,
            ):
                for vc in range(0 if skip_logits else NVC):
                    w = min(512, VP - vc * 512)
                    if vc == 0:
                        ett = et0
                    else:
                        ett = etpool.tile([P, DT, 512], BF16, tag="et")
                        nc.scalar.dma_start(
                            ett[:, :, :w],
                            et[:].rearrange("(d p) v -> p d v", p=P)[
                                :, :, ds(vc * 512, w)
                            ],
                        )
                    for st in range(ST):
                        psum = ps_l.tile([P, 512], F32, tag="lmm")
                        for d in range(DT):
                            nc.tensor.matmul(
                                psum[:, :w],
                                xb[:, d, ts(st, P)],
                                ett[:, d, :w],
                                start=(d == 0),
                                stop=(d == DT - 1),
                            )
                        ot = opool.tile([P, 512], F32, tag="o")
                        nc.vector.tensor_copy(ot[:, :w], psum[:, :w])
                        # alternate the two HWDGE queues: ~51MB of logit
                        # writes would otherwise serialize on one queue
                        eng = nc.sync if st % 2 == 0 else nc.scalar
                        eng.dma_start(
                            out[ts(st, P), ds(vc * 512, w)], ot[:, :w]
                        )

    nc.compile()
    return nc


def make_host_inputs(input_ids, tok_emb, pos_emb, Wqkvs, Wos, S, VP):
    """Shard/transpose/cast inputs on host. Returns per-core in_maps."""
    ids = np.asarray(input_ids).reshape(-1)[:S]
    tok = np.asarray(tok_emb, dtype=np.float32)
    pos = np.asarray(pos_emb, dtype=np.float32)

    x0 = tok[ids]                                  # (S, H) fp32
    x0f = np.ascontiguousarray(x0.T)               # (H, S)
    x0b = x0f.astype(BF)
    peT = np.ascontiguousarray(pos[:S].T).astype(BF)
    x0peT = np.ascontiguousarray((x0 + pos[:S]).T).astype(BF)

    # causal 0/1 masks in ST layout for the 4 diagonal-region offsets
    msk = np.zeros((4, P, 512), np.float32)
    tri = (np.arange(P)[:, None] <= np.arange(P)[None, :]).astype(np.float32)
    for o in range(4):
        for j in range(4):
            blk = msk[o, :, j * P : (j + 1) * P]
            if j > o:
                blk[:] = 1.0
            elif j == o:
                blk[:] = tri
    msk = msk.astype(BF)

    # padded vocab shards of the (transposed) embedding
    tokP = np.zeros((NCORES * VP, H), np.float32)
    tokP[:V] = tok

    in_maps = []
    for r in range(NCORES):
        wq_l = []
        for Wqkv in Wqkvs:
            Wq = Wqkv[r * P : (r + 1) * P]
            Wk = Wqkv[H + r * P : H + (r + 1) * P]
            Wv = Wqkv[2 * H + r * P : 2 * H + (r + 1) * P]
            wq_l.append(np.concatenate([Wq, Wk, Wv], axis=0).T)  # (H, 384)
        wqkv_arr = np.stack(wq_l).astype(BF)
        wo_arr = np.stack([np.asarray(Wo).T for Wo in Wos]).astype(BF)
        eT = np.ascontiguousarray(tokP[r * VP : (r + 1) * VP].T).astype(BF)
        in_maps.append(
            {
                "x0f": x0f,
                "x0b": x0b,
                "pe": peT,
                "x0pe": x0peT,
                "wqkv": np.ascontiguousarray(wqkv_arr),
                "wo": np.ascontiguousarray(wo_arr),
                "et": eT,
                "msk": msk,
            }
        )
    return in_maps


_NC_CACHE = {}


def _get_nc(S, VP):
    key = (S, VP)
    if key not in _NC_CACHE:
        _NC_CACHE[key] = build_nc(S, VP)
    return _NC_CACHE[key]


class _Runner:
    """Compiled 8-core PJRT executable for one Bass program.

    Mirrors bass2jax.run_bass_via_pjrt's multi-core path but keeps the
    jitted function so repeated calls (benchmarking) skip recompilation.
    """

    def __init__(self, nc):
        import jax
        import jax.numpy as jnp
        from jax.sharding import Mesh, PartitionSpec, NamedSharding
        from jax.experimental.shard_map import shard_map
        from concourse import bass2jax
        from concourse import mybir as _mybir

        bass2jax.install_neuronx_cc_hook()
        self.jax = jax
        self.jnp = jnp
        self.nc = nc

        partition_name = (
            nc.partition_id_tensor.name if nc.partition_id_tensor else None
        )
        in_names, out_names, out_avals = [], [], []
        for alloc in nc.m.functions[0].allocations:
            if not isinstance(alloc, _mybir.MemoryLocationSet):
                continue
            name = alloc.memorylocations[0].name
            if alloc.kind == "ExternalInput":
                if name != partition_name:
                    in_names.append(name)
            elif alloc.kind == "ExternalOutput":
                out_names.append(name)
                out_avals.append(
                    jax.core.ShapedArray(
                        tuple(alloc.tensor_shape), _mybir.dt.np(alloc.dtype)
                    )
                )
        n_params = len(in_names)
        all_in_names = list(in_names) + list(out_names)
        if partition_name is not None:
            all_in_names.append(partition_name)
        self.in_names = in_names
        self.out_names = out_names
        self.out_avals = out_avals

        def _body(*args):
            operands = list(args)
            if partition_name is not None:
                operands.append(bass2jax.partition_id_tensor())
            outs = bass2jax._bass_exec_p.bind(
                *operands,
                out_avals=tuple(out_avals),
                in_names=tuple(all_in_names),
                out_names=tuple(out_names),
                lowering_input_output_aliases=(),
                sim_require_finite=True,
                sim_require_nnan=True,
                nc=nc,
            )
            return tuple(outs)

        devices = jax.devices()[:NCORES]
        self.mesh = Mesh(np.asarray(devices), ("core",))
        self.sharding = NamedSharding(self.mesh, PartitionSpec("core"))
        n_outs = len(out_names)
        donate = tuple(range(n_params, n_params + n_outs))
        self.fn = jax.jit(
            shard_map(
                _body,
                mesh=self.mesh,
                in_specs=(PartitionSpec("core"),) * (n_params + n_outs),
                out_specs=(PartitionSpec("core"),) * n_outs,
                check_rep=False,
            ),
            donate_argnums=donate,
            keep_unused=True,
        )
        self.dev_inputs = None

    def stage_inputs(self, in_maps):
        """device_put concatenated per-core inputs (reusable across calls)."""
        concat = [
            np.concatenate([np.asarray(m[n]) for m in in_maps], axis=0)
            for n in self.in_names
        ]
        self.dev_inputs = [
            self.jax.device_put(a, self.sharding) for a in concat
        ]

    def _zeros(self):
        if not hasattr(self, "_zeros_fn"):
            jnp = self.jnp
            shapes = [
                ((NCORES * av.shape[0],) + tuple(av.shape[1:]), av.dtype)
                for av in self.out_avals
            ]
            self._zeros_fn = self.jax.jit(
                lambda: tuple(jnp.zeros(s, d) for s, d in shapes),
                out_shardings=tuple(self.sharding for _ in shapes),
            )
        outs = self._zeros_fn()
        self.jax.block_until_ready(outs)
        return outs

    def run(self, zeros=None):
        if zeros is None:
            zeros = self._zeros()
        out = self.fn(*self.dev_inputs, *zeros)
        self.jax.block_until_ready(out)
        return out

    def results(self, out_arrs):
        per_core = []
        for c in range(NCORES):
            d = {}
            for i, name in enumerate(self.out_names):
                a = np.asarray(out_arrs[i])
                d[name] = a.reshape((NCORES,) + self.out_avals[i].shape)[c]
            per_core.append(d)
        return per_core


_RUNNER_CACHE = {}


def _get_runner(S, VP):
    key = (S, VP)
    if key not in _RUNNER_CACHE:
        _RUNNER_CACHE[key] = _Runner(_get_nc(S, VP))
    return _RUNNER_CACHE[key]


def run_model(input_ids, tok_emb, pos_emb, Wqkvs, Wos, S=S_FULL, VP=VP_FULL,
              bench_iters=0):
    import time as _time

    runner = _get_runner(S, VP)
    in_maps = make_host_inputs(input_ids, tok_emb, pos_emb, Wqkvs, Wos, S, VP)
    runner.stage_inputs(in_maps)
    out = runner.run()
    times = []
    for _ in range(bench_iters):
        zs = runner._zeros()
        t0 = _time.perf_counter()
        out = runner.run(zeros=zs)
        times.append(_time.perf_counter() - t0)
    res = runner.results(out)
    logits = np.concatenate(
        [res[r]["logits"] for r in range(NCORES)], axis=1
    )[:, : min(V, NCORES * VP)]
    return logits, times


def kernel(**inputs):
    logits, _ = run_model(
        inputs["input_ids"],
        inputs["tok_emb"],
        inputs["pos_emb"],
        [np.asarray(inputs["Wqkv0"], np.float32),
         np.asarray(inputs["Wqkv1"], np.float32)],
        [np.asarray(inputs["Wo0"], np.float32),
         np.asarray(inputs["Wo1"], np.float32)],
    )
    return logits[None].astype(np.float32)



# revision 22
# speedup vs baseline: 1.2027x; 1.1837x over previous
"""MiniGPT (2-layer causal transformer + tied-embedding logits) on 8 trn2 cores.

Sharding:
  - Attention: tensor-parallel over heads (2 heads/core). Each core computes
    q,k,v projections for its 2 heads, causal-blocked attention in "ST"
    (scores-transposed) layout, and the normalized per-head combined output
    (128 of the 1024 combined dims). A per-chunk AllGather concatenates the
    per-core combined slices; every core then applies the FULL output
    projection and residual locally (redundant compute, but it removes the
    second per-layer collective round - collectives serialize on the
    collective engine at ~15us fixed cost each, so 4 AGs/layer beats 8).
  - Logits: vocab-sharded tied-embedding matmul. Core r holds rows
    [r*VP, (r+1)*VP) of the (zero-padded) token embedding and emits
    logits[:, r*VP:(r+1)*VP]; host concatenates and trims padding.
    The logits matmul runs in 3-term double-fp8: x and E are each split
    into an fp8e4m3 value+residual pair at one shared power-of-2 scale
    (64), and x*E ~ x8*E8 + x8*Er + xr*E8 accumulates in one fp32 PSUM
    group of DoubleRow matmuls (0.5 cycles/row) - 0.75x the PE cycles
    of bf16 at slightly BETTER accuracy (the pair carries ~2^-10
    relative precision vs bf16's 2^-9; measured l2 1.0e-3 vs 2.0e-3).
    The residual state xT is kept in a 64x-scaled frame (x0f and Wo are
    pre-scaled on host) so x8 is a plain fp8 copy of xT and the final
    PSUM carries logits*4096, undone exactly on host.

Attention matmuls run in bf16 (fp32 PSUM accumulation); softmax runs in
fp32 (scores are tiny, ~1e-4, so exp needs no max-subtraction; masked
entries are exactly zeroed by a multiplicative 0/1 mask after exp).

Activations live transposed (xT: [hidden, seq]) the whole time, which makes
every matmul PE-friendly with no on-device transposes at all.
"""

import os as _os
import sys as _sys

if "jax" not in _sys.modules and _os.environ.get("JAX_PLATFORMS") == "cpu":
    # bass2jax needs the axon trn2 devices; a cpu pin would hide them
    del _os.environ["JAX_PLATFORMS"]

import numpy as np
import ml_dtypes

import concourse.bass as bass
import concourse.mybir as mybir
import concourse.tile as tile
from concourse import bacc
from concourse.bass import ts, ds
from concourse.bass_utils import run_bass_kernel_spmd

P = 128
H = 1024
DT = H // P  # 8 hidden-dim tiles
NH = 16
HD = 64
V = 50259
NCORES = 8
S_FULL = 2048
VP_FULL = -(-V // NCORES)  # 6283 per-core padded vocab shard

F32 = mybir.dt.float32
BF16 = mybir.dt.bfloat16
F8 = mybir.dt.float8e4
BF = ml_dtypes.bfloat16
F8NP = ml_dtypes.float8_e4m3
XSCALE = 64.0  # shared power-of-2 fp8 frame for x and E (logits path)


def build_nc(S, VP, no_collective=False, skip_layers=False, skip_logits=False):
    """Build the per-core Bass program (SPMD: same NEFF on all 8 cores).

    no_collective=True replaces the AllGather with local DMA block copies
    (single-core cost-model profiling only - numerically wrong)."""
    ST = S // P       # seq tiles of 128
    SC = S // 512     # seq chunks of 512
    NVC = -(-VP // 512)  # vocab chunks

    nc = bacc.Bacc("TRN2", target_bir_lowering=False, debug=False,
                   num_devices=NCORES)

    # --- DRAM I/O (per-core) ---
    x0f = nc.dram_tensor("x0f", [H, S], F32, kind="ExternalInput")
    x0b = nc.dram_tensor("x0b", [H, S], BF16, kind="ExternalInput")
    pe = nc.dram_tensor("pe", [H, S], BF16, kind="ExternalInput")
    x0pe = nc.dram_tensor("x0pe", [H, S], BF16, kind="ExternalInput")
    wqkv = nc.dram_tensor("wqkv", [2, H, 3 * P], BF16, kind="ExternalInput")
    wo = nc.dram_tensor("wo", [2, H, H], BF16, kind="ExternalInput")
    et8 = nc.dram_tensor("et8", [H, VP], F8, kind="ExternalInput")
    etr = nc.dram_tensor("etr", [H, VP], F8, kind="ExternalInput")
    msk = nc.dram_tensor("msk", [4, P, 512], BF16, kind="ExternalInput")
    out = nc.dram_tensor("logits", [S, VP], F32, kind="ExternalOutput")

    Exp = mybir.ActivationFunctionType.Exp
    Copy = mybir.ActivationFunctionType.Copy
    DblRow = mybir.MatmulPerfMode.DoubleRow

    with tile.TileContext(nc) as tc:
        with (
            tc.tile_pool(name="const", bufs=1) as const,
            tc.tile_pool(name="dram", bufs=1, space="DRAM") as dram,
        ):
            # persistent SBUF tensors; xT lives in the 64x-scaled frame
            xT = const.tile([P, DT, S], F32, tag="xT")      # fp32 residual
            xb = const.tile([P, DT, S], BF16, tag="xb")     # bf16 x, true scale
            maskt = const.tile([P, 4, 512], BF16, tag="maskt")
            # final x as fp8 value+residual pair, staged via DRAM so the
            # logits-scope SBUF tiles don't coexist with the layer pools
            x8d = dram.tile([P, DT, S], F8, tag="x8d")
            xrd = dram.tile([P, DT, S], F8, tag="xrd")


            with (
                tc.tile_pool(name="big", bufs=2) as big,
                tc.tile_pool(name="lay", bufs=1) as lay,
                tc.tile_pool(name="pestream", bufs=4) as pestream,
                tc.tile_pool(name="xpepool", bufs=4) as xpepool,
                tc.tile_pool(name="epool", bufs=6) as epool,
                tc.tile_pool(name="bcpool", bufs=2) as bcpool,
                tc.tile_pool(name="ps_a", bufs=4, space="PSUM") as ps_a,
                tc.tile_pool(name="ps_pv", bufs=2, space="PSUM") as ps_pv,
            ):
                wq_tiles = {}
                pe0 = lay.tile([P, DT, 512], BF16, tag="pe0")

                def load_wq(l):
                    t = lay.tile([P, DT, 3 * P], BF16, tag=f"wqkv{l}")
                    nc.sync.dma_start(
                        t[:], wqkv[l].rearrange("(d p) e -> p d e", p=P)
                    )
                    wq_tiles[l] = t

                load_wq(0)
                for layer in range(0 if skip_layers else 2):
                    wq = wq_tiles[layer]

                    # fused q,k projections over streamed xpe=(xb+pe) tiles
                    qT = lay.tile([P, S], BF16, tag="qT")
                    kT = lay.tile([P, S], BF16, tag="kT")
                    for c in range(SC):
                        psQ = ps_a.tile([P, 512], F32, tag="mm")
                        psK = ps_a.tile([P, 512], F32, tag="mm")
                        for d in range(DT):
                            xpet = xpepool.tile([P, 512], BF16, tag="xpet")
                            if layer == 0:
                                nc.sync.dma_start(
                                    xpet[:],
                                    x0pe[:].rearrange(
                                        "(d p) s -> p d s", p=P
                                    )[:, d, ts(c, 512)],
                                )
                            else:
                                if c == 0:
                                    pet = pe0[:, d]
                                else:
                                    pet = pestream.tile(
                                        [P, 512], BF16, tag="pet"
                                    )
                                    nc.sync.dma_start(
                                        pet[:],
                                        pe[:].rearrange(
                                            "(d p) s -> p d s", p=P
                                        )[:, d, ts(c, 512)],
                                    )
                                nc.vector.tensor_add(
                                    xpet[:], xb[:, d, ts(c, 512)], pet[:]
                                )
                            nc.tensor.matmul(
                                psQ[:], wq[:, d, ts(0, P)], xpet[:],
                                start=(d == 0), stop=(d == DT - 1),
                                skip_group_check=True,
                            )
                            nc.tensor.matmul(
                                psK[:], wq[:, d, ts(1, P)], xpet[:],
                                start=(d == 0), stop=(d == DT - 1),
                                skip_group_check=True,
                            )
                        nc.vector.tensor_copy(qT[:, ts(c, 512)], psQ[:])
                        nc.vector.tensor_copy(kT[:, ts(c, 512)], psK[:])

                    if layer == 0:
                        # xb load deferred: q/k proj reads x0pe, so this
                        # streams in under the projection matmuls
                        for d in range(DT):
                            nc.sync.dma_start(
                                xb[:, d],
                                x0b[:].rearrange("(d p) s -> p d s", p=P)[:, d],
                            )

                    # v projection -> natural layout [ks, vdim] with ones cols:
                    # v[:, st, 0:65]  = [vA(64) | 1]   (head A lhsT)
                    # v[:, st, 65:130]= [vB(64) | 1]   (head B lhsT)
                    v = lay.tile([P, ST, 130], BF16, tag="v")
                    nc.vector.memset(v[:, :, 64:65], 1.0)
                    nc.vector.memset(v[:, :, 129:130], 1.0)
                    for st in range(ST):
                        psum = ps_a.tile([P, 512], F32, tag="mm")
                        for d in range(DT):
                            nc.tensor.matmul(
                                psum[:, :P],
                                xb[:, d, ts(st, P)],
                                wq[:, d, 2 * P : 3 * P],
                                start=(d == 0),
                                stop=(d == DT - 1),
                            )
                        nc.vector.tensor_copy(v[:, st, 0:64], psum[:, 0:64])
                        nc.vector.tensor_copy(
                            v[:, st, 65:129], psum[:, 64:128]
                        )

                    # full WoT for this layer (2MB): issued here so it
                    # streams in under the attention chunks; first needed at
                    # the output projection
                    woL = lay.tile([P, DT, H], BF16, tag="woL")
                    nc.sync.dma_start(
                        woL[:], wo[layer].rearrange("(d p) e -> p d e", p=P)
                    )

                    if layer == 0:
                        # deferred loads: masks are first needed by the
                        # attention exps, the fp32 residual baseline by the
                        # layer-0 residual add - keep them off the startup
                        # DMA critical path; xT is chunked so chunk 0 lands
                        # before the first residual add
                        nc.sync.dma_start(
                            maskt[:], msk[:].rearrange("o p n -> p o n")
                        )
                        for c in range(SC):
                            nc.sync.dma_start(
                                xT[:, :, ts(c, 512)],
                                x0f[:].rearrange("(d p) s -> p d s", p=P)[
                                    :, :, ts(c, 512)
                                ],
                            )

                    # causal attention, ST layout, 2 heads packed on partitions
                    # comb layout [64, 2, S]: [:,0]=head A dims, [:,1]=head B
                    comb = lay.tile([64, 2, S], BF16, tag="comb")
                    ag_outs = []

                    def ag_emit(qc, comb=comb):
                        agin = dram.tile([P, 512], BF16, tag=f"agin{qc}")
                        agout = dram.tile(
                            [NCORES * P, 512], BF16, tag=f"agout{qc}",
                            addr_space="Local" if no_collective else "Shared",
                        )
                        nc.sync.dma_start(
                            agin[:].rearrange("(h p) s -> p h s", p=64),
                            comb[:, :, ts(qc, 512)],
                        )
                        if no_collective:
                            for r in range(NCORES):
                                nc.sync.dma_start(
                                    agout[r * P : (r + 1) * P, :], agin[:]
                                )
                        else:
                            nc.gpsimd.collective_compute(
                                "AllGather",
                                mybir.AluOpType.bypass,
                                ins=[agin[:]],
                                outs=[agout[:]],
                                replica_groups=[list(range(NCORES))],
                            )
                        ag_outs.append(agout)

                    for qc in range(SC):
                        n_kt = 4 * (qc + 1)
                        pvA = ps_pv.tile([65, 512], F32, tag="pvA")
                        pvB = ps_pv.tile([65, 512], F32, tag="pvB")
                        for kt in range(n_kt):
                            sA = ps_a.tile([P, 512], F32, tag="mm")
                            sB = ps_a.tile([P, 512], F32, tag="mm")
                            nc.tensor.matmul(
                                sA[:], kT[0:64, ts(kt, P)],
                                qT[0:64, ts(qc, 512)],
                                start=True, stop=True,
                            )
                            nc.tensor.matmul(
                                sB[:], kT[64:P, ts(kt, P)],
                                qT[64:P, ts(qc, 512)],
                                start=True, stop=True,
                            )
                            eA = epool.tile([P, 512], BF16, tag="e")
                            eB = epool.tile([P, 512], BF16, tag="e")
                            nc.scalar.activation(eA[:], sA[:], Exp, scale=0.125)
                            nc.scalar.activation(eB[:], sB[:], Exp, scale=0.125)
                            o = kt - 4 * qc
                            if o >= 0:
                                nc.vector.tensor_mul(eA[:], eA[:], maskt[:, o])
                                nc.vector.tensor_mul(eB[:], eB[:], maskt[:, o])
                            nc.tensor.matmul(
                                pvA[:], v[:, kt, 0:65], eA[:],
                                start=(kt == 0), stop=(kt == n_kt - 1),
                                skip_group_check=True,
                            )
                            nc.tensor.matmul(
                                pvB[:], v[:, kt, 65:130], eB[:],
                                start=(kt == 0), stop=(kt == n_kt - 1),
                                skip_group_check=True,
                            )
                        # normalize: pv psum row 64 is the softmax denominator
                        bcA = bcpool.tile([64, 512], F32, tag="bcA")
                        bcB = bcpool.tile([64, 512], F32, tag="bcB")
                        nc.vector.reciprocal(bcA[0:1, :], pvA[64:65, :])
                        nc.vector.reciprocal(bcB[0:1, :], pvB[64:65, :])
                        nc.gpsimd.partition_broadcast(bcA[:], bcA[0:1, :])
                        nc.gpsimd.partition_broadcast(bcB[:], bcB[0:1, :])
                        nc.vector.tensor_mul(
                            comb[:, 0, ts(qc, 512)], pvA[0:64, :], bcA[:]
                        )
                        nc.vector.tensor_mul(
                            comb[:, 1, ts(qc, 512)], pvB[0:64, :], bcB[:]
                        )
                        ag_emit(qc)

                    # per-chunk AllGather (issued inside the qc loop via
                    # ag_emit) so comm overlaps remaining attention chunks
                    agouts = ag_outs

                    if layer == 0:
                        # prefetches issued while the SP queue would otherwise
                        # idle at the comb-AG waits (in-order queue): layer-1
                        # qkv weights and layer-1's first pe chunk
                        load_wq(1)
                        nc.sync.dma_start(
                            pe0[:],
                            pe[:].rearrange("(d p) s -> p d s", p=P)[
                                :, :, ts(0, 512)
                            ],
                        )

                    # full output projection on every core (redundant across
                    # cores but collective-free: the comb AG already gave
                    # every core the full combined tensor, so computing all
                    # 1024 e-dims locally replaces the delta-AllGather round
                    # trip), fused with the residual update per 512-chunk
                    for c in range(SC):
                        cb = big.tile([P, DT, 512], BF16, tag="cb")
                        nc.sync.dma_start(
                            cb[:],
                            agouts[c][:].rearrange("(d p) s -> p d s", p=P),
                        )
                        for e in range(DT):
                            psum = ps_a.tile([P, 512], F32, tag="mm")
                            for d in range(DT):
                                nc.tensor.matmul(
                                    psum[:],
                                    woL[:, d, ts(e, P)],
                                    cb[:, d, :],
                                    start=(d == 0),
                                    stop=(d == DT - 1),
                                )
                            nc.vector.tensor_add(
                                xT[:, e, ts(c, 512)], xT[:, e, ts(c, 512)],
                                psum[:],
                            )
                            if layer == 0:
                                # layer 1 consumes x in true scale as bf16
                                nc.scalar.activation(
                                    xb[:, e, ts(c, 512)],
                                    xT[:, e, ts(c, 512)],
                                    Copy, scale=1.0 / XSCALE,
                                )
                            else:
                                # final x: fp8 value+residual pair (already
                                # in the 64x frame), staged out to DRAM for
                                # the logits phase
                                x8c = bcpool.tile([P, 512], F8, tag="x8c")
                                nc.scalar.activation(
                                    x8c[:], xT[:, e, ts(c, 512)], Copy,
                                )
                                xrc = bcpool.tile([P, 512], F8, tag="xrc")
                                nc.vector.tensor_sub(
                                    xrc[:], xT[:, e, ts(c, 512)], x8c[:]
                                )
                                nc.sync.dma_start(
                                    x8d[:, e, ts(c, 512)], x8c[:]
                                )
                                nc.sync.dma_start(
                                    xrd[:, e, ts(c, 512)], xrc[:]
                                )

            # logits: 3-term double-fp8 x.T @ E_shard.T, vocab-chunked.
            # x*E*4096 = x8*E8 + x8*Er + xr*E8 (xr*Er dropped, <=2^-8 rel);
            # all 12 DoubleRow matmuls accumulate into one PSUM group and
            # the 2^-12 is undone exactly on host.
            with (
                tc.tile_pool(name="xqpool", bufs=1) as xqpool,
                tc.tile_pool(name="etpool", bufs=3) as etpool,
                tc.tile_pool(name="opool", bufs=4) as opool,
                tc.tile_pool(name="ps_l", bufs=8, space="PSUM") as ps_l,
            ):
                x8f = xqpool.tile([P, DT, S], F8, tag="x8f")
                xrf = xqpool.tile([P, DT, S], F8, tag="xrf")
                for c in range(SC):
                    nc.sync.dma_start(
                        x8f[:, :, ts(c, 512)], x8d[:, :, ts(c, 512)]
                    )
                    nc.sync.dma_start(
                        xrf[:, :, ts(c, 512)], xrd[:, :, ts(c, 512)]
                    )
                for vc in range(0 if skip_logits else NVC):
                    w = min(512, VP - vc * 512)
                    et8t = etpool.tile([P, DT, 512], F8, tag="et8")
                    etrt = etpool.tile([P, DT, 512], F8, tag="etr")
                    nc.scalar.dma_start(
                        et8t[:, :, :w],
                        et8[:].rearrange("(d p) v -> p d v", p=P)[
                            :, :, ds(vc * 512, w)
                        ],
                    )
                    nc.scalar.dma_start(
                        etrt[:, :, :w],
                        etr[:].rearrange("(d p) v -> p d v", p=P)[
                            :, :, ds(vc * 512, w)
                        ],
                    )
                    for st in range(ST):
                        psum = ps_l.tile([P, 512], F32, tag="lmm")
                        terms = ((x8f, et8t), (x8f, etrt), (xrf, et8t))
                        for ti, (lh, rh) in enumerate(terms):
                            for dp in range(DT // 2):
                                nc.tensor.matmul(
                                    psum[:, :w],
                                    lh[:, 2 * dp : 2 * dp + 2, ts(st, P)],
                                    rh[:, 2 * dp : 2 * dp + 2, :w],
                                    start=(ti == 0 and dp == 0),
                                    stop=(ti == 2 and dp == DT // 2 - 1),
                                    perf_mode=DblRow,
                                    skip_group_check=True,
                                )
                        ot = opool.tile([P, 512], F32, tag="o")
                        nc.vector.tensor_copy(ot[:, :w], psum[:, :w])
                        # alternate the two HWDGE queues: ~51MB of logit
                        # writes would otherwise serialize on one queue
                        eng = nc.sync if st % 2 == 0 else nc.scalar
                        eng.dma_start(
                            out[ts(st, P), ds(vc * 512, w)], ot[:, :w]
                        )

    nc.compile()
    return nc


def make_host_inputs(input_ids, tok_emb, pos_emb, Wqkvs, Wos, S, VP):
    """Shard/transpose/cast inputs on host. Returns per-core in_maps."""
    ids = np.asarray(input_ids).reshape(-1)[:S]
    tok = np.asarray(tok_emb, dtype=np.float32)
    pos = np.asarray(pos_emb, dtype=np.float32)

    x0 = tok[ids]                                  # (S, H) fp32
    x0T = np.ascontiguousarray(x0.T)               # (H, S)
    x0f = x0T * np.float32(XSCALE)                 # 64x frame for fp8 logits
    x0b = x0T.astype(BF)
    peT = np.ascontiguousarray(pos[:S].T).astype(BF)
    x0peT = np.ascontiguousarray((x0 + pos[:S]).T).astype(BF)

    # causal 0/1 masks in ST layout for the 4 diagonal-region offsets
    msk = np.zeros((4, P, 512), np.float32)
    tri = (np.arange(P)[:, None] <= np.arange(P)[None, :]).astype(np.float32)
    for o in range(4):
        for j in range(4):
            blk = msk[o, :, j * P : (j + 1) * P]
            if j > o:
                blk[:] = 1.0
            elif j == o:
                blk[:] = tri
    msk = msk.astype(BF)

    # padded vocab shards of the (transposed) embedding
    tokP = np.zeros((NCORES * VP, H), np.float32)
    tokP[:V] = tok

    in_maps = []
    for r in range(NCORES):
        wq_l = []
        for Wqkv in Wqkvs:
            Wq = Wqkv[r * P : (r + 1) * P]
            Wk = Wqkv[H + r * P : H + (r + 1) * P]
            Wv = Wqkv[2 * H + r * P : 2 * H + (r + 1) * P]
            wq_l.append(np.concatenate([Wq, Wk, Wv], axis=0).T)  # (H, 384)
        wqkv_arr = np.stack(wq_l).astype(BF)
        # Wo pre-scaled so oproj deltas land in xT's 64x frame
        wo_arr = np.stack(
            [np.asarray(Wo).T * np.float32(XSCALE) for Wo in Wos]
        ).astype(BF)
        # embedding shard as fp8 value+residual pair in the 64x frame
        esT = np.ascontiguousarray(
            tokP[r * VP : (r + 1) * VP].T
        ) * np.float32(XSCALE)
        e8 = esT.astype(F8NP)
        er = (esT - e8.astype(np.float32)).astype(F8NP)
        in_maps.append(
            {
                "x0f": x0f,
                "x0b": x0b,
                "pe": peT,
                "x0pe": x0peT,
                "wqkv": np.ascontiguousarray(wqkv_arr),
                "wo": np.ascontiguousarray(wo_arr),
                "et8": e8,
                "etr": er,
                "msk": msk,
            }
        )
    return in_maps


_NC_CACHE = {}


def _get_nc(S, VP):
    key = (S, VP)
    if key not in _NC_CACHE:
        _NC_CACHE[key] = build_nc(S, VP)
    return _NC_CACHE[key]


class _Runner:
    """Compiled 8-core PJRT executable for one Bass program.

    Mirrors bass2jax.run_bass_via_pjrt's multi-core path but keeps the
    jitted function so repeated calls (benchmarking) skip recompilation.
    """

    def __init__(self, nc):
        import jax
        import jax.numpy as jnp
        from jax.sharding import Mesh, PartitionSpec, NamedSharding
        from jax.experimental.shard_map import shard_map
        from concourse import bass2jax
        from concourse import mybir as _mybir

        bass2jax.install_neuronx_cc_hook()
        self.jax = jax
        self.jnp = jnp
        self.nc = nc

        partition_name = (
            nc.partition_id_tensor.name if nc.partition_id_tensor else None
        )
        in_names, out_names, out_avals = [], [], []
        for alloc in nc.m.functions[0].allocations:
            if not isinstance(alloc, _mybir.MemoryLocationSet):
                continue
            name = alloc.memorylocations[0].name
            if alloc.kind == "ExternalInput":
                if name != partition_name:
                    in_names.append(name)
            elif alloc.kind == "ExternalOutput":
                out_names.append(name)
                out_avals.append(
                    jax.core.ShapedArray(
                        tuple(alloc.tensor_shape), _mybir.dt.np(alloc.dtype)
                    )
                )
        n_params = len(in_names)
        all_in_names = list(in_names) + list(out_names)
        if partition_name is not None:
            all_in_names.append(partition_name)
        self.in_names = in_names
        self.out_names = out_names
        self.out_avals = out_avals

        def _body(*args):
            operands = list(args)
            if partition_name is not None:
                operands.append(bass2jax.partition_id_tensor())
            outs = bass2jax._bass_exec_p.bind(
                *operands,
                out_avals=tuple(out_avals),
                in_names=tuple(all_in_names),
                out_names=tuple(out_names),
                lowering_input_output_aliases=(),
                sim_require_finite=True,
                sim_require_nnan=True,
                nc=nc,
            )
            return tuple(outs)

        devices = jax.devices()[:NCORES]
        self.mesh = Mesh(np.asarray(devices), ("core",))
        self.sharding = NamedSharding(self.mesh, PartitionSpec("core"))
        n_outs = len(out_names)
        donate = tuple(range(n_params, n_params + n_outs))
        self.fn = jax.jit(
            shard_map(
                _body,
                mesh=self.mesh,
                in_specs=(PartitionSpec("core"),) * (n_params + n_outs),
                out_specs=(PartitionSpec("core"),) * n_outs,
                check_rep=False,
            ),
            donate_argnums=donate,
            keep_unused=True,
        )
        self.dev_inputs = None

    def stage_inputs(self, in_maps):
        """device_put concatenated per-core inputs (reusable across calls)."""
        concat = [
            np.concatenate([np.asarray(m[n]) for m in in_maps], axis=0)
            for n in self.in_names
        ]
        self.dev_inputs = [
            self.jax.device_put(a, self.sharding) for a in concat
        ]

    def _zeros(self):
        if not hasattr(self, "_zeros_fn"):
            jnp = self.jnp
            shapes = [
                ((NCORES * av.shape[0],) + tuple(av.shape[1:]), av.dtype)
                for av in self.out_avals
            ]
            self._zeros_fn = self.jax.jit(
                lambda: tuple(jnp.zeros(s, d) for s, d in shapes),
                out_shardings=tuple(self.sharding for _ in shapes),
            )
        outs = self._zeros_fn()
        self.jax.block_until_ready(outs)
        return outs

    def run(self, zeros=None):
        if zeros is None:
            zeros = self._zeros()
        out = self.fn(*self.dev_inputs, *zeros)
        self.jax.block_until_ready(out)
        return out

    def results(self, out_arrs):
        per_core = []
        for c in range(NCORES):
            d = {}
            for i, name in enumerate(self.out_names):
                a = np.asarray(out_arrs[i])
                d[name] = a.reshape((NCORES,) + self.out_avals[i].shape)[c]
            per_core.append(d)
        return per_core


_RUNNER_CACHE = {}


def _get_runner(S, VP):
    key = (S, VP)
    if key not in _RUNNER_CACHE:
        _RUNNER_CACHE[key] = _Runner(_get_nc(S, VP))
    return _RUNNER_CACHE[key]


def run_model(input_ids, tok_emb, pos_emb, Wqkvs, Wos, S=S_FULL, VP=VP_FULL,
              bench_iters=0):
    import time as _time

    runner = _get_runner(S, VP)
    in_maps = make_host_inputs(input_ids, tok_emb, pos_emb, Wqkvs, Wos, S, VP)
    runner.stage_inputs(in_maps)
    out = runner.run()
    times = []
    for _ in range(bench_iters):
        zs = runner._zeros()
        t0 = _time.perf_counter()
        out = runner.run(zeros=zs)
        times.append(_time.perf_counter() - t0)
    res = runner.results(out)
    logits = np.concatenate(
        [res[r]["logits"] for r in range(NCORES)], axis=1
    )[:, : min(V, NCORES * VP)]
    # device PSUM carries logits * XSCALE^2 (exact power of 2) - undo here
    logits = logits * np.float32(1.0 / (XSCALE * XSCALE))
    return logits, times


def kernel(**inputs):
    logits, _ = run_model(
        inputs["input_ids"],
        inputs["tok_emb"],
        inputs["pos_emb"],
        [np.asarray(inputs["Wqkv0"], np.float32),
         np.asarray(inputs["Wqkv1"], np.float32)],
        [np.asarray(inputs["Wo0"], np.float32),
         np.asarray(inputs["Wo1"], np.float32)],
    )
    return logits[None].astype(np.float32)

